# revision 1
# baseline (speedup 1.0000x reference)
"""DiffusionConv (4x GCN message passing) Trainium2 kernel, 8-core SPMD.

Strategy: shard destination nodes across 8 cores (3750 each). Each core:
  - gathers source-node feature rows (fp16) for its edges via dma_gather
    (4 SWDGE queues round-robin, ~10-chunk blocks, deep buffer rotation),
    edges pre-sorted by destination and padded per 128-dst window,
  - builds banded edge-weight matrices stT[e, dstcol, chunk] on DVE with
    all operands packed in the innermost dim (4x DVE mode),
  - aggregates with operand-swapped matmuls (lhsT=msg f-slice, rhs=band)
    accumulating y^T[f, d] directly in PSUM (no PE transposes),
  - applies the 32x32 weight matrices per window via block-diagonal
    matmuls on the y^T tiles, adds bias, writes f32.
No cross-core communication: each core reads a full replica of x.
"""
import sys, os
for p in ('/opt/trn_rl_repo', '/root/.axon_site/_ro/trn_rl_repo'):
    if os.path.isdir(p) and p not in sys.path:
        sys.path.insert(0, p)

import numpy as np
import ml_dtypes

N = 30000
C = 32
T = 12
ES = C * T          # 384, feature row width
E = 480000
NCORES = 8
ND = N // NCORES    # 3750 dst nodes per core
NDP = 3840          # padded to 30 windows of 128
WIN = 128           # dst window width
NWIN = NDP // WIN   # 30
ADJS = ("fwd1", "fwd2", "bck1", "bck2")

bf16 = np.float16 if os.environ.get("K_DT", "fp16") == "fp16" else ml_dtypes.bfloat16


def _prep_edges(ei, ew):
    """Split one adjacency's edges by destination core and sort by dst."""
    src = np.asarray(ei[0]).astype(np.int64)
    dst = np.asarray(ei[1]).astype(np.int64)
    w = np.asarray(ew).astype(np.float32)
    core = dst // ND
    out = []
    for k in range(NCORES):
        sel = core == k
        dl = dst[sel] - k * ND
        s = src[sel]
        wv = w[sel]
        order = np.argsort(dl, kind="stable")
        dl = dl[order]
        s = s[order]
        wv = wv[order]
        win = dl // WIN
        bounds = np.searchsorted(win, np.arange(NWIN + 1))
        counts = np.diff(bounds)
        out.append((s, dl, wv, bounds, counts))
    return out


def _build_adj_inputs(per_core):
    """Uniform-shape device inputs for one adjacency: gather idx tiles and
    per-edge (dst-local column, weight) tiles; per-window chunk counts are
    shared across cores (max)."""
    ncw = np.maximum(1, -(-np.stack([pc[4] for pc in per_core], 0).max(0) // 128))
    nch = int(ncw.sum())
    epad = nch * 128
    winid = np.repeat(np.arange(NWIN), ncw * 128)
    gis, dlcs = [], []
    for k in range(NCORES):
        s, dl, wv, bounds, counts = per_core[k]
        srcp = np.zeros(epad, np.int64)
        dlp = np.zeros(epad, np.int64)
        ewp = np.zeros(epad, np.float32)
        off = 0
        for w in range(NWIN):
            c = int(counts[w])
            lo, hi = int(bounds[w]), int(bounds[w + 1])
            srcp[off:off + c] = s[lo:hi]
            dlp[off:off + c] = dl[lo:hi]
            ewp[off:off + c] = wv[lo:hi]
            dlp[off + c:off + int(ncw[w]) * 128] = w * WIN  # pads (ew 0)
            off += int(ncw[w]) * 128
        # gather idx, wrapped layout [128, epad//16]: idx i -> [i%16, i//16]
        gi = np.tile(srcp.astype(np.int16).reshape(-1, 16).T, (8, 1))
        col = (dlp - winid * WIN).astype(np.float32)   # 0..127, exact in fp16
        # device layout [128 partitions = edge-in-chunk, nch]
        dlc = col.reshape(nch, 128).T.astype(bf16)
        ewc = ewp.reshape(nch, 128).T.astype(bf16)
        gis.append(np.ascontiguousarray(gi))
        dlcs.append(np.ascontiguousarray(np.stack([dlc, ewc], 1)))  # [128, 2, nch]
    return ncw.astype(np.int64), gis, dlcs


def _build_B(Ws):
    """Block-diagonal weight tiles B[a][k]: [128, 384] fp16.
    B[a,k][p, c'*12 + t] = W_a[c, c'] with (t, c) = divmod(128k + p, 32)."""
    B = np.zeros((4, 3, 128, ES), np.float32)
    for a in range(4):
        Wa = np.asarray(Ws[a]).astype(np.float32)
        for k in range(3):
            phi = 128 * k + np.arange(128)
            t = phi // 32
            c = phi % 32
            B[a, k, np.arange(128)[:, None], np.arange(32)[None, :] * 12 + t[:, None]] = Wa[c, :]
    return B.astype(bf16)


_CACHE = {}
LAST_RESULTS = None
LAST_NC = None
LAST_NCWS = None
LAST_INMAPS = None


def _get_program(ncws, rep=1, ablate=()):
    """Build (and cache) the Bass program for given per-adjacency window
    chunk counts. ncws: tuple of 4 tuples of NWIN ints. rep>1 repeats the
    pipeline (device-time measurement via slope); ablate disables stages
    for profiling ("gatheronly", "nogather", "nowphase", "nostbuild").
    """
    key = (ncws, rep, tuple(sorted(ablate)))
    if key in _CACHE:
        return _CACHE[key]

    import concourse.mybir as mybir
    import concourse.tile as tile
    from concourse import bacc

    DT = (mybir.dt.float16 if os.environ.get("K_DT", "fp16") == "fp16"
          else mybir.dt.bfloat16)
    nchs = [int(sum(ncw)) for ncw in ncws]
    max_nch = max(int(v) for ncw in ncws for v in ncw)

    nc = bacc.Bacc("TRN2", target_bir_lowering=False, debug=False,
                   num_devices=NCORES, num_swdge_queues=4)
    x_d = nc.dram_tensor("xrows", [N, ES], DT, kind="ExternalInput")
    gi_d = [nc.dram_tensor(f"gi_{a}", [128, nchs[ai] * 8], mybir.dt.int16,
                           kind="ExternalInput") for ai, a in enumerate(ADJS)]
    dew_d = [nc.dram_tensor(f"dew_{a}", [128, 2, nchs[ai]], DT,
                            kind="ExternalInput") for ai, a in enumerate(ADJS)]
    B_d = nc.dram_tensor("B", [4, 3, 128, ES], DT, kind="ExternalInput")
    bias_d = nc.dram_tensor("biasrep", [128, ES], mybir.dt.float32, kind="ExternalInput")
    iotaN_d = nc.dram_tensor("iotaN", [128, 128 * max_nch], DT, kind="ExternalInput")
    out_d = nc.dram_tensor("out", [NDP, ES], mybir.dt.float32, kind="ExternalOutput")

    gq = [0]  # gather queue rotation counter
    GBLK = int(os.environ.get('K_GBLK', 10))   # chunks per gather block
    GBUFS = int(os.environ.get('K_GBUFS', 10))  # msg buffers in flight

    with tile.TileContext(nc) as tc:
        with tc.tile_pool(name="const", bufs=1) as cpool, \
             tc.tile_pool(name="work", bufs=3) as wpool, \
             tc.tile_pool(name="yt", bufs=2) as ytpool, \
             tc.tile_pool(name="outsb", bufs=2) as opool, \
             tc.tile_pool(name="psagg", bufs=4, space="PSUM") as ps_agg, \
             tc.tile_pool(name="pstp", bufs=2, space="PSUM") as ps_tp, \
             tc.tile_pool(name="psout", bufs=2, space="PSUM") as ps_out:

            B_t = cpool.tile([128, 4, 3, ES], DT)
            nc.sync.dma_start(out=B_t[:], in_=B_d.ap().rearrange("a k p e -> p a k e"))
            bias_t = cpool.tile([128, ES], mybir.dt.float32)
            nc.sync.dma_start(out=bias_t[:], in_=bias_d.ap())
            iotaN_t = cpool.tile([128, 128, max_nch], DT)
            nc.sync.dma_start(out=iotaN_t[:], in_=iotaN_d.ap().rearrange(
                "p (j c) -> p j c", c=max_nch))
            from concourse.masks import make_identity
            ident = cpool.tile([128, 128], DT)
            make_identity(nc, ident[:])
            dew_t = []
            gi_t = []
            for ai, a in enumerate(ADJS):
                dt_ = cpool.tile([128, 2, nchs[ai]], DT, name=f"dewt_{a}")
                nc.sync.dma_start(out=dt_[:], in_=dew_d[ai].ap())
                dew_t.append(dt_)
                gt = cpool.tile([128, nchs[ai] * 8], mybir.dt.int16, name=f"git_{a}")
                nc.sync.dma_start(out=gt[:], in_=gi_d[ai].ap())
                gi_t.append(gt)

            last_msg = [None]
            for _rep in range(rep):
                choffs = [0, 0, 0, 0]
                for w in range(NWIN):
                    yTs = []
                    for ai in range(4):
                        nch = int(ncws[ai][w])
                        choff = choffs[ai]
                        # banded matrix build, transposed layout
                        # stT[e, j, c] = (dlc[e,c] == j) * ew[e,c]
                        stT = wpool.tile([128, 128, max_nch], DT, tag="st", bufs=6)
                        if "nostbuild" in ablate and "gatheronly" not in ablate:
                            nc.vector.tensor_copy(out=stT[:, :, :nch],
                                                  in_=iotaN_t[:, :, :nch])
                        if "nostbuild" not in ablate and "gatheronly" not in ablate:
                            dlc_bT = dew_t[ai][:, 0, choff:choff + nch] \
                                .rearrange("p (o c) -> p o c", o=1) \
                                .to_broadcast([128, 128, nch])
                            ewc_bT = dew_t[ai][:, 1, choff:choff + nch] \
                                .rearrange("p (o c) -> p o c", o=1) \
                                .to_broadcast([128, 128, nch])
                            nc.vector.tensor_tensor(
                                out=stT[:, :, :nch], in0=iotaN_t[:, :, :nch],
                                in1=dlc_bT, op=mybir.AluOpType.is_equal)
                            nc.vector.tensor_tensor(
                                out=stT[:, :, :nch], in0=stT[:, :, :nch],
                                in1=ewc_bT, op=mybir.AluOpType.mult)
                        # gather + aggregation matmuls (y[d, f] in PSUM)
                        psy = ps_agg.tile([128, ES], mybir.dt.float32,
                                          tag="agg")
                        for b0 in range(0, nch, GBLK):
                            bn = min(GBLK, nch - b0)
                            msg = wpool.tile([128, GBLK, ES], DT,
                                             tag="msg", bufs=GBUFS)
                            if "nogather" not in ablate:
                                nc.gpsimd.dma_gather(
                                    msg[:, :bn, :], x_d.ap(),
                                    gi_t[ai][:, (choff + b0) * 8:(choff + b0 + bn) * 8],
                                    bn * 128, bn * 128, ES, elem_step=ES,
                                    single_packet=False, queue_num=gq[0] % 4)
                                gq[0] += 1
                                last_msg[0] = msg
                            else:
                                nc.vector.memset(msg[:, :bn, 0:2], 0.0)
                            if "gatheronly" in ablate:
                                continue
                            for jj in range(bn):
                                ch = b0 + jj
                                nc.tensor.matmul(
                                    psy[:, :],
                                    lhsT=stT[:, :, ch],
                                    rhs=msg[:, jj, :],
                                    start=(ch == 0), stop=(ch == nch - 1))
                        choffs[ai] = choff + nch
                        if "gatheronly" in ablate:
                            continue
                        ysb = ytpool.tile([128, ES], DT, tag=f"ysb{ai}")
                        nc.scalar.copy(out=ysb[:], in_=psy[:])
                        yTs.append(ysb)
                    # ---- W transform for this window ----
                    if "nowphase" in ablate or "gatheronly" in ablate:
                        continue
                    yTt = []
                    for i in range(12):
                        ai, k = divmod(i, 3)
                        pst = ps_tp.tile([128, 128], DT, tag="tp")
                        nc.tensor.transpose(
                            pst[:], yTs[ai][:, 128 * k:128 * (k + 1)], ident[:])
                        yT = ytpool.tile([128, 128], DT, tag="yT", bufs=13)
                        if i % 2 == 0:
                            nc.vector.tensor_copy(out=yT[:], in_=pst[:])
                        else:
                            nc.scalar.copy(out=yT[:], in_=pst[:])
                        yTt.append(yT)
                    pso = ps_out.tile([128, ES], mybir.dt.float32, tag="wout")
                    for i in range(12):
                        ai, k = divmod(i, 3)
                        nc.tensor.matmul(pso[:], lhsT=yTt[i][:],
                                         rhs=B_t[:, ai, k, :],
                                         start=(i == 0), stop=(i == 11))
                    outsb = opool.tile([128, ES], mybir.dt.float32, tag="out")
                    nc.vector.tensor_tensor(out=outsb[:], in0=pso[:], in1=bias_t[:],
                                            op=mybir.AluOpType.add)
                    nc.sync.dma_start(out=out_d.ap()[128 * w:128 * (w + 1), :],
                                      in_=outsb[:])
                if "gatheronly" in ablate:
                    # chain program completion to the last gather
                    outsb = opool.tile([128, ES], mybir.dt.float32, tag="out")
                    nc.vector.tensor_copy(out=outsb[:, 0:GBLK * ES // 128],
                                          in_=last_msg[0][:, 0, 0:GBLK * ES // 128])
                    nc.sync.dma_start(out=out_d.ap()[0:128, :], in_=outsb[:])

    nc.compile()
    _CACHE[key] = nc
    return nc


def _host_prep(x, Ws, bias, eis, ews):
    x_rows = np.ascontiguousarray(
        np.asarray(x).astype(np.float32).transpose(0, 2, 1).reshape(N, ES)).astype(bf16)
    ncws, gis, dlcs = [], [], []
    for a in ADJS:
        pc = _prep_edges(np.asarray(eis[a]), np.asarray(ews[a]))
        ncw, gi, dew = _build_adj_inputs(pc)
        ncws.append(tuple(int(v) for v in ncw))
        gis.append(gi)
        dlcs.append(dew)
    B = _build_B(Ws)
    bias_rep = np.ascontiguousarray(
        np.tile(np.repeat(np.asarray(bias).astype(np.float32), T)[None, :], (128, 1)))
    max_nch = max(int(v) for ncw in ncws for v in ncw)
    iotaN = np.ascontiguousarray(np.broadcast_to(
        np.repeat(np.arange(128, dtype=np.float32), max_nch)[None, :],
        (128, 128 * max_nch))).astype(bf16)
    in_maps = []
    for k in range(NCORES):
        m = {"xrows": x_rows, "B": B, "biasrep": bias_rep, "iotaN": iotaN}
        for ai, a in enumerate(ADJS):
            m[f"gi_{a}"] = gis[ai][k]
            m[f"dew_{a}"] = dlcs[ai][k]
        in_maps.append(m)
    return tuple(ncws), in_maps


def kernel(x, W_fwd1, W_fwd2, W_bck1, W_bck2, bias,
           ew_fwd1, ew_fwd2, ew_bck1, ew_bck2,
           ei_fwd1, ei_fwd2, ei_bck1, ei_bck2):
    from concourse.bass_utils import run_bass_kernel_spmd

    x = np.asarray(x)
    eis = dict(fwd1=ei_fwd1, fwd2=ei_fwd2, bck1=ei_bck1, bck2=ei_bck2)
    ews = dict(fwd1=ew_fwd1, fwd2=ew_fwd2, bck1=ew_bck1, bck2=ew_bck2)
    Ws = [W_fwd1, W_fwd2, W_bck1, W_bck2]

    ncws, in_maps = _host_prep(x, Ws, bias, eis, ews)
    nc = _get_program(ncws)

    res = run_bass_kernel_spmd(nc, in_maps, core_ids=list(range(NCORES)))
    global LAST_RESULTS, LAST_NC, LAST_INMAPS, LAST_NCWS
    LAST_RESULTS = res
    LAST_NC = nc
    LAST_INMAPS = in_maps
    LAST_NCWS = ncws

    out = np.empty((N, C, T), np.float32)
    for k in range(NCORES):
        shard = res.results[k]["out"][:ND]           # [3750, 384], phi'=c*12+t
        out[k * ND:(k + 1) * ND] = shard.reshape(ND, C, T)
    return out



# revision 16
# speedup vs baseline: 1.6931x; 1.6931x over previous
"""DiffusionConv (4x GCN message passing) Trainium2 kernel, 8-core SPMD.

Strategy: shard destination nodes across 8 cores (3750 each). Each core:
  - gathers source-node feature rows (fp16) for its edges via dma_gather
    (4 SWDGE queues round-robin, ~10-chunk blocks, deep buffer rotation),
    edges pre-sorted by destination and padded per 128-dst window,
  - builds banded edge-weight matrices stT[e, dstcol, chunk] on DVE with
    all operands packed in the innermost dim (4x DVE mode),
  - aggregates with operand-swapped matmuls (lhsT=msg f-slice, rhs=band)
    accumulating y^T[f, d] directly in PSUM (no PE transposes),
  - applies the 32x32 weight matrices per window via block-diagonal
    matmuls on the y^T tiles, adds bias, writes f32.
No cross-core communication: each core reads a full replica of x.
"""
import sys, os
for p in ('/opt/trn_rl_repo', '/root/.axon_site/_ro/trn_rl_repo'):
    if os.path.isdir(p) and p not in sys.path:
        sys.path.insert(0, p)

import numpy as np
import ml_dtypes

N = 30000
C = 32
T = 12
ES = C * T          # 384, feature row width
E = 480000
NCORES = 8
ND = N // NCORES    # 3750 dst nodes per core
NDP = 3840          # padded to 30 windows of 128
WIN = 128           # dst window width
NWIN = NDP // WIN   # 30
ADJS = ("fwd1", "fwd2", "bck1", "bck2")

bf16 = np.float16 if os.environ.get("K_DT", "fp16") == "fp16" else ml_dtypes.bfloat16
# x-row storage: fp8e3 (e3m4) packed into one 512B DMA packet per row.
# The gather path is packet-rate-limited (512B max packet); fp16 rows are
# 768B = 2 packets. e3m4 keeps rel err ~1.5e-2 < 2e-2 tolerance.
XDT_S = os.environ.get("K_XDT", "fp8e3")
XW = 512 if XDT_S == "fp8e3" else ES          # padded row width (elements)
x8t = ml_dtypes.float8_e3m4


def _prep_edges(ei, ew):
    """Split one adjacency's edges by destination core and sort by dst."""
    src = np.asarray(ei[0]).astype(np.int64)
    dst = np.asarray(ei[1]).astype(np.int64)
    w = np.asarray(ew).astype(np.float32)
    core = dst // ND
    out = []
    for k in range(NCORES):
        sel = core == k
        dl = dst[sel] - k * ND
        s = src[sel]
        wv = w[sel]
        order = np.argsort(dl, kind="stable")
        dl = dl[order]
        s = s[order]
        wv = wv[order]
        win = dl // WIN
        bounds = np.searchsorted(win, np.arange(NWIN + 1))
        counts = np.diff(bounds)
        out.append((s, dl, wv, bounds, counts))
    return out


def _build_adj_inputs(per_core):
    """Uniform-shape device inputs for one adjacency: gather idx tiles and
    per-edge (dst-local column, weight) tiles; per-window chunk counts are
    shared across cores (max)."""
    ncw = np.maximum(1, -(-np.stack([pc[4] for pc in per_core], 0).max(0) // 128))
    nch = int(ncw.sum())
    epad = nch * 128
    winid = np.repeat(np.arange(NWIN), ncw * 128)
    gis, dlcs = [], []
    for k in range(NCORES):
        s, dl, wv, bounds, counts = per_core[k]
        srcp = np.zeros(epad, np.int64)
        dlp = np.zeros(epad, np.int64)
        ewp = np.zeros(epad, np.float32)
        off = 0
        srcsort = os.environ.get("K_SRCSORT", "1") == "1"
        for w in range(NWIN):
            c = int(counts[w])
            lo, hi = int(bounds[w]), int(bounds[w + 1])
            sw, dw, ww = s[lo:hi], dl[lo:hi], wv[lo:hi]
            if srcsort:
                so = np.argsort(sw, kind="stable")
                sw, dw, ww = sw[so], dw[so], ww[so]
            srcp[off:off + c] = sw
            dlp[off:off + c] = dw
            ewp[off:off + c] = ww
            dlp[off + c:off + int(ncw[w]) * 128] = w * WIN  # pads (ew 0)
            off += int(ncw[w]) * 128
        # gather idx, wrapped layout [128, epad//16]: idx i -> [i%16, i//16]
        gi = np.tile(srcp.astype(np.int16).reshape(-1, 16).T, (8, 1))
        col = (dlp - winid * WIN).astype(np.float32)   # 0..127, exact in fp16
        # device layout [128 partitions = edge-in-chunk, nch]
        dlc = col.reshape(nch, 128).T.astype(bf16)
        ewc = ewp.reshape(nch, 128).T.astype(bf16)
        gis.append(np.ascontiguousarray(gi))
        dlcs.append(np.ascontiguousarray(np.stack([dlc, ewc], 1)))  # [128, 2, nch]
    return ncw.astype(np.int64), gis, dlcs


def _build_B(Ws):
    """Block-diagonal weight tiles B[a][k]: [128, 384] fp16.
    B[a,k][p, c'*12 + t] = W_a[c, c'] with (t, c) = divmod(128k + p, 32)."""
    B = np.zeros((4, 3, 128, ES), np.float32)
    for a in range(4):
        Wa = np.asarray(Ws[a]).astype(np.float32)
        for k in range(3):
            phi = 128 * k + np.arange(128)
            t = phi // 32
            c = phi % 32
            B[a, k, np.arange(128)[:, None], np.arange(32)[None, :] * 12 + t[:, None]] = Wa[c, :]
    return B.astype(bf16)


_CACHE = {}
LAST_RESULTS = None
LAST_NC = None
LAST_NCWS = None
LAST_INMAPS = None


def _get_program(ncws, rep=1, ablate=(), ges=None):
    """Build (and cache) the Bass program for given per-adjacency window
    chunk counts. ncws: tuple of 4 tuples of NWIN ints. rep>1 repeats the
    pipeline (device-time measurement via slope); ablate disables stages
    for profiling ("gatheronly", "nogather", "nowphase", "nostbuild").
    ges: gather elem_size in elements (timing experiments; only correct
    when == XW, the default).
    """
    if ges is None:
        ges = XW
    gblk = int(os.environ.get('K_GBLK', 10))
    gbufs = int(os.environ.get('K_GBUFS', 10))
    key = (ncws, rep, tuple(sorted(ablate)), ges, gblk, gbufs)
    if key in _CACHE:
        return _CACHE[key]

    import concourse.mybir as mybir
    import concourse.tile as tile
    from concourse import bacc

    DT = (mybir.dt.float16 if os.environ.get("K_DT", "fp16") == "fp16"
          else mybir.dt.bfloat16)
    XDT = mybir.dt.float8e3 if XDT_S == "fp8e3" else DT
    nchs = [int(sum(ncw)) for ncw in ncws]
    max_nch = max(int(v) for ncw in ncws for v in ncw)

    nc = bacc.Bacc("TRN2", target_bir_lowering=False, debug=False,
                   num_devices=NCORES, num_swdge_queues=4)
    x_d = nc.dram_tensor("xrows", [N, ges], XDT, kind="ExternalInput")
    gi_d = [nc.dram_tensor(f"gi_{a}", [128, nchs[ai] * 8], mybir.dt.int16,
                           kind="ExternalInput") for ai, a in enumerate(ADJS)]
    dew_d = [nc.dram_tensor(f"dew_{a}", [128, 2, nchs[ai]], DT,
                            kind="ExternalInput") for ai, a in enumerate(ADJS)]
    B_d = nc.dram_tensor("B", [4, 3, 128, ES], DT, kind="ExternalInput")
    bias_d = nc.dram_tensor("biasrep", [128, ES], mybir.dt.float32, kind="ExternalInput")
    iotaN_d = nc.dram_tensor("iotaN", [128, 128 * max_nch], DT, kind="ExternalInput")
    out_d = nc.dram_tensor("out", [NDP, ES], mybir.dt.float32, kind="ExternalOutput")

    gq = [0]  # gather queue rotation counter
    GBLK = gblk   # chunks per gather block
    GBUFS = gbufs  # msg buffers in flight

    with tile.TileContext(nc) as tc:
        with tc.tile_pool(name="const", bufs=1) as cpool, \
             tc.tile_pool(name="work", bufs=3) as wpool, \
             tc.tile_pool(name="yt", bufs=2) as ytpool, \
             tc.tile_pool(name="outsb", bufs=2) as opool, \
             tc.tile_pool(name="psagg", bufs=4, space="PSUM") as ps_agg, \
             tc.tile_pool(name="pstp", bufs=2, space="PSUM") as ps_tp, \
             tc.tile_pool(name="psout", bufs=2, space="PSUM") as ps_out:

            B_t = cpool.tile([128, 4, 3, ES], DT)
            nc.sync.dma_start(out=B_t[:], in_=B_d.ap().rearrange("a k p e -> p a k e"))
            bias_t = cpool.tile([128, ES], mybir.dt.float32)
            nc.sync.dma_start(out=bias_t[:], in_=bias_d.ap())
            iotaN_t = cpool.tile([128, 128, max_nch], DT)
            nc.sync.dma_start(out=iotaN_t[:], in_=iotaN_d.ap().rearrange(
                "p (j c) -> p j c", c=max_nch))
            from concourse.masks import make_identity
            ident = cpool.tile([128, 128], DT)
            make_identity(nc, ident[:])
            dew_t = []
            gi_t = []
            for ai, a in enumerate(ADJS):
                dt_ = cpool.tile([128, 2, nchs[ai]], DT, name=f"dewt_{a}")
                nc.sync.dma_start(out=dt_[:], in_=dew_d[ai].ap())
                dew_t.append(dt_)
                gt = cpool.tile([128, nchs[ai] * 8], mybir.dt.int16, name=f"git_{a}")
                nc.sync.dma_start(out=gt[:], in_=gi_d[ai].ap())
                gi_t.append(gt)

            last_msg = [None]
            for _rep in range(rep):
                choffs = [0, 0, 0, 0]
                for w in range(NWIN):
                    yTs = []
                    for ai in range(4):
                        nch = int(ncws[ai][w])
                        choff = choffs[ai]
                        # banded matrix build, transposed layout
                        # stT[e, j, c] = (dlc[e,c] == j) * ew[e,c]
                        stT = wpool.tile([128, 128, max_nch], DT, tag="st", bufs=6)
                        if "nostbuild" in ablate and "gatheronly" not in ablate:
                            nc.vector.tensor_copy(out=stT[:, :, :nch],
                                                  in_=iotaN_t[:, :, :nch])
                        if "nostbuild" not in ablate and "gatheronly" not in ablate:
                            dlc_bT = dew_t[ai][:, 0, choff:choff + nch] \
                                .rearrange("p (o c) -> p o c", o=1) \
                                .to_broadcast([128, 128, nch])
                            ewc_bT = dew_t[ai][:, 1, choff:choff + nch] \
                                .rearrange("p (o c) -> p o c", o=1) \
                                .to_broadcast([128, 128, nch])
                            nc.vector.tensor_tensor(
                                out=stT[:, :, :nch], in0=iotaN_t[:, :, :nch],
                                in1=dlc_bT, op=mybir.AluOpType.is_equal)
                            nc.vector.tensor_tensor(
                                out=stT[:, :, :nch], in0=stT[:, :, :nch],
                                in1=ewc_bT, op=mybir.AluOpType.mult)
                        # gather + aggregation matmuls (y[d, f] in PSUM)
                        psy = ps_agg.tile([128, ES], mybir.dt.float32,
                                          tag="agg")
                        for b0 in range(0, nch, GBLK):
                            bn = min(GBLK, nch - b0)
                            msg = wpool.tile([128, GBLK, ges], XDT,
                                             tag="msg", bufs=GBUFS)
                            if "nogather" not in ablate:
                                nc.gpsimd.dma_gather(
                                    msg[:, :bn, :], x_d.ap(),
                                    gi_t[ai][:, (choff + b0) * 8:(choff + b0 + bn) * 8],
                                    bn * 128, bn * 128, ges, elem_step=ges,
                                    single_packet=False, queue_num=gq[0] % 4)
                                gq[0] += 1
                                last_msg[0] = msg
                            else:
                                nc.vector.memset(msg[:, :bn, 0:2], 0.0)
                            if "gatheronly" in ablate:
                                continue
                            for jj in range(bn):
                                ch = b0 + jj
                                nc.tensor.matmul(
                                    psy[:, :],
                                    lhsT=stT[:, :, ch],
                                    rhs=msg[:, jj, :ES],
                                    start=(ch == 0), stop=(ch == nch - 1))
                        choffs[ai] = choff + nch
                        if "gatheronly" in ablate:
                            continue
                        ysb = ytpool.tile([128, ES], DT, tag=f"ysb{ai}")
                        nc.scalar.copy(out=ysb[:], in_=psy[:])
                        yTs.append(ysb)
                    # ---- W transform for this window ----
                    if "nowphase" in ablate or "gatheronly" in ablate:
                        continue
                    yTt = []
                    for i in range(12):
                        ai, k = divmod(i, 3)
                        pst = ps_tp.tile([128, 128], DT, tag="tp")
                        nc.tensor.transpose(
                            pst[:], yTs[ai][:, 128 * k:128 * (k + 1)], ident[:])
                        yT = ytpool.tile([128, 128], DT, tag="yT", bufs=13)
                        if i % 2 == 0:
                            nc.vector.tensor_copy(out=yT[:], in_=pst[:])
                        else:
                            nc.scalar.copy(out=yT[:], in_=pst[:])
                        yTt.append(yT)
                    pso = ps_out.tile([128, ES], mybir.dt.float32, tag="wout")
                    for i in range(12):
                        ai, k = divmod(i, 3)
                        nc.tensor.matmul(pso[:], lhsT=yTt[i][:],
                                         rhs=B_t[:, ai, k, :],
                                         start=(i == 0), stop=(i == 11))
                    outsb = opool.tile([128, ES], mybir.dt.float32, tag="out")
                    nc.vector.tensor_tensor(out=outsb[:], in0=pso[:], in1=bias_t[:],
                                            op=mybir.AluOpType.add)
                    nc.sync.dma_start(out=out_d.ap()[128 * w:128 * (w + 1), :],
                                      in_=outsb[:])
                if "gatheronly" in ablate:
                    # chain program completion to the last gather
                    outsb = opool.tile([128, ES], mybir.dt.float32, tag="out")
                    nc.vector.tensor_copy(out=outsb[:, 0:GBLK * ES // 128],
                                          in_=last_msg[0][:, 0, 0:GBLK * ES // 128])
                    nc.sync.dma_start(out=out_d.ap()[0:128, :], in_=outsb[:])

    nc.compile()
    _CACHE[key] = nc
    return nc


def _host_prep(x, Ws, bias, eis, ews):
    xr = np.asarray(x).astype(np.float32).transpose(0, 2, 1).reshape(N, ES)
    if XDT_S == "fp8e3":
        x_rows = np.zeros((N, XW), x8t)
        x_rows[:, :ES] = xr.astype(x8t)
    else:
        x_rows = np.ascontiguousarray(xr).astype(bf16)
    ncws, gis, dlcs = [], [], []
    for a in ADJS:
        pc = _prep_edges(np.asarray(eis[a]), np.asarray(ews[a]))
        ncw, gi, dew = _build_adj_inputs(pc)
        ncws.append(tuple(int(v) for v in ncw))
        gis.append(gi)
        dlcs.append(dew)
    B = _build_B(Ws)
    bias_rep = np.ascontiguousarray(
        np.tile(np.repeat(np.asarray(bias).astype(np.float32), T)[None, :], (128, 1)))
    max_nch = max(int(v) for ncw in ncws for v in ncw)
    iotaN = np.ascontiguousarray(np.broadcast_to(
        np.repeat(np.arange(128, dtype=np.float32), max_nch)[None, :],
        (128, 128 * max_nch))).astype(bf16)
    in_maps = []
    for k in range(NCORES):
        m = {"xrows": x_rows, "B": B, "biasrep": bias_rep, "iotaN": iotaN}
        for ai, a in enumerate(ADJS):
            m[f"gi_{a}"] = gis[ai][k]
            m[f"dew_{a}"] = dlcs[ai][k]
        in_maps.append(m)
    return tuple(ncws), in_maps


def kernel(x, W_fwd1, W_fwd2, W_bck1, W_bck2, bias,
           ew_fwd1, ew_fwd2, ew_bck1, ew_bck2,
           ei_fwd1, ei_fwd2, ei_bck1, ei_bck2):
    from concourse.bass_utils import run_bass_kernel_spmd

    x = np.asarray(x)
    eis = dict(fwd1=ei_fwd1, fwd2=ei_fwd2, bck1=ei_bck1, bck2=ei_bck2)
    ews = dict(fwd1=ew_fwd1, fwd2=ew_fwd2, bck1=ew_bck1, bck2=ew_bck2)
    Ws = [W_fwd1, W_fwd2, W_bck1, W_bck2]

    ncws, in_maps = _host_prep(x, Ws, bias, eis, ews)
    nc = _get_program(ncws)

    res = run_bass_kernel_spmd(nc, in_maps, core_ids=list(range(NCORES)))
    global LAST_RESULTS, LAST_NC, LAST_INMAPS, LAST_NCWS
    LAST_RESULTS = res
    LAST_NC = nc
    LAST_INMAPS = in_maps
    LAST_NCWS = ncws

    out = np.empty((N, C, T), np.float32)
    for k in range(NCORES):
        shard = res.results[k]["out"][:ND]           # [3750, 384], phi'=c*12+t
        out[k * ND:(k + 1) * ND] = shard.reshape(ND, C, T)
    return out



# revision 36
# speedup vs baseline: 2.6363x; 1.5571x over previous
"""DiffusionConv (4x GCN message passing) Trainium2 kernel, 8-core SPMD.

Strategy: shard destination nodes across 8 cores. The per-edge gather of
source rows is packet-rate limited (~2ns per 512B DMA packet), so x rows
are stored as fp8 e3m4 padded to one 512B packet each (fp16 rows = 768B =
2 packets; e3m4 keeps rel err ~1.5e-2 < 2e-2). Each core:
  - gathers source-node rows for its edges via dma_gather (4 SWDGE queues
    round-robin, 8-chunk blocks, 12-buffer rotation); edges are grouped
    by 128-dst window, sorted by src inside a window (HBM locality), with
    a core-invariant valid count per window (num_idxs_reg is shared by the
    SPMD program); slots beyond it use idx -1, which the gather engine
    skips without emitting packets,
  - dst nodes are assigned to (core, window) bins by a host-side greedy
    balancer so per-window chunk counts match across cores (kills the
    max-over-cores padding),
  - builds banded edge-weight matrices stT[e, dstcol, chunk] on DVE,
  - aggregates y[d, f] per adjacency in PSUM via matmuls (lhsT=band fp16,
    rhs=msg fp8e3 - mixed operand dtypes),
  - applies the 32x32 weight matrices per window via PE transposes +
    block-diagonal matmuls, adds bias, writes fp16 (host upcasts to f32).
No cross-core communication: each core reads a full replica of x.
"""
import sys, os
for p in ('/opt/trn_rl_repo', '/root/.axon_site/_ro/trn_rl_repo'):
    if os.path.isdir(p) and p not in sys.path:
        sys.path.insert(0, p)

import numpy as np
import ml_dtypes

N = 30000
C = 32
T = 12
ES = C * T          # 384, feature row width
E = 480000
NCORES = 8
ND = N // NCORES    # 3750 dst nodes per core
NDP = 3840          # padded to 30 windows of 128
WIN = 128           # dst window width
NWIN = NDP // WIN   # 30
ADJS = ("fwd1", "fwd2", "bck1", "bck2")

bf16 = np.float16 if os.environ.get("K_DT", "fp16") == "fp16" else ml_dtypes.bfloat16
# x-row storage: fp8e3 (e3m4) packed into one 512B DMA packet per row.
# The gather path is packet-rate-limited (512B max packet); fp16 rows are
# 768B = 2 packets. e3m4 keeps rel err ~1.5e-2 < 2e-2 tolerance.
XDT_S = os.environ.get("K_XDT", "fp8e3")
XW = 512 if XDT_S == "fp8e3" else ES          # padded row width (elements)
x8t = ml_dtypes.float8_e3m4


def _dst_mapping(eis):
    """Assign dst nodes to the 240 (core, window) bins, balancing each
    adjacency's per-bin edge count (greedy vector LPT, <=128 nodes/bin).
    Cuts the chunk padding that comes from taking the max count over cores
    per window. Returns (binof, colof): node -> bin id, column in window."""
    NB = NCORES * NWIN
    deg = np.zeros((N, 4), np.int64)
    for ai, a in enumerate(ADJS):
        dst = np.asarray(eis[a][1]).astype(np.int64)
        deg[:, ai] = np.bincount(dst, minlength=N)
    order = np.argsort(-deg.sum(1), kind="stable")
    loads = np.zeros((NB, 4), np.float64)
    cnt = np.zeros(NB, np.int64)
    binof = np.empty(N, np.int32)
    colof = np.empty(N, np.int32)
    for n in order:
        d = deg[n]
        score = np.max(loads + d, axis=1) + 1e-3 * loads.sum(1)
        score[cnt >= WIN] = np.inf
        b = int(np.argmin(score))
        binof[n] = b
        colof[n] = cnt[b]
        cnt[b] += 1
        loads[b] += d
    return binof, colof


def _prep_edges(ei, ew, binof, colof):
    """Split one adjacency's edges by destination core and sort by dst."""
    src = np.asarray(ei[0]).astype(np.int64)
    dst = np.asarray(ei[1]).astype(np.int64)
    w = np.asarray(ew).astype(np.float32)
    dbin = binof[dst]
    core = dbin // NWIN
    dloc = (dbin % NWIN) * WIN + colof[dst]   # dst-local row in [0, NDP)
    out = []
    for k in range(NCORES):
        sel = core == k
        dl = dloc[sel]
        s = src[sel]
        wv = w[sel]
        order = np.argsort(dl, kind="stable")
        dl = dl[order]
        s = s[order]
        wv = wv[order]
        win = dl // WIN
        bounds = np.searchsorted(win, np.arange(NWIN + 1))
        counts = np.diff(bounds)
        out.append((s, dl, wv, bounds, counts))
    return out


def _build_adj_inputs(per_core):
    """Uniform-shape device inputs for one adjacency: gather idx tiles and
    per-edge (dst-local column, weight) tiles; per-window chunk counts are
    shared across cores (max).

    Each (window) has a shared valid count V = max_k count: cores with fewer
    real edges get (V - count) dummy gathers (src 0, ew 0); the remaining
    slots up to the 128-multiple boundary are skip-pads (src -1) that the
    gather engine drops without emitting DMA packets. num_idxs_reg must
    equal the per-instruction valid count, so V must be core-invariant."""
    cnts = np.stack([pc[4] for pc in per_core], 0)      # [NCORES, NWIN]
    vws = np.maximum(1, cnts.max(0))                    # shared valid counts
    padskip = os.environ.get("K_PADSKIP", "1") == "1"
    ncw = -(-vws // 128)
    nch = int(ncw.sum())
    epad = nch * 128
    winid = np.repeat(np.arange(NWIN), ncw * 128)
    gis, dlcs = [], []
    for k in range(NCORES):
        s, dl, wv, bounds, counts = per_core[k]
        srcp = np.zeros(epad, np.int64)
        dlp = np.zeros(epad, np.int64)
        ewp = np.zeros(epad, np.float32)
        off = 0
        srcsort = os.environ.get("K_SRCSORT", "1") == "1"
        for w in range(NWIN):
            c = int(counts[w])
            v = int(vws[w])
            lo, hi = int(bounds[w]), int(bounds[w + 1])
            sw, dw, ww = s[lo:hi], dl[lo:hi], wv[lo:hi]
            if srcsort:
                so = np.argsort(sw, kind="stable")
                sw, dw, ww = sw[so], dw[so], ww[so]
            srcp[off:off + c] = sw
            dlp[off:off + c] = dw
            ewp[off:off + c] = ww
            # dummy gathers up to the shared valid count V (src 0, ew 0)
            srcp[off + c:off + v] = 0
            dlp[off + c:off + v] = w * WIN
            # skip-pads to the chunk boundary: src -1 -> no DMA packets;
            # msg rows keep stale finite data and the band column is 0
            srcp[off + v:off + int(ncw[w]) * 128] = -1 if padskip else 0
            dlp[off + v:off + int(ncw[w]) * 128] = w * WIN
            off += int(ncw[w]) * 128
        # gather idx, wrapped layout [128, epad//16]: idx i -> [i%16, i//16]
        gi = np.tile(srcp.astype(np.int16).reshape(-1, 16).T, (8, 1))
        col = (dlp - winid * WIN).astype(np.float32)   # 0..127, exact in fp16
        # device layout [128 partitions = edge-in-chunk, nch]
        dlc = col.reshape(nch, 128).T.astype(bf16)
        ewc = ewp.reshape(nch, 128).T.astype(bf16)
        gis.append(np.ascontiguousarray(gi))
        dlcs.append(np.ascontiguousarray(np.stack([dlc, ewc], 1)))  # [128, 2, nch]
    if not padskip:
        vws = ncw * 128
    return np.stack([ncw, vws]).astype(np.int64), gis, dlcs


def _build_B(Ws):
    """Block-diagonal weight tiles B[a][k]: [128, 384] fp16.
    B[a,k][p, c'*12 + t] = W_a[c, c'] with (t, c) = divmod(128k + p, 32)."""
    B = np.zeros((4, 3, 128, ES), np.float32)
    for a in range(4):
        Wa = np.asarray(Ws[a]).astype(np.float32)
        for k in range(3):
            phi = 128 * k + np.arange(128)
            t = phi // 32
            c = phi % 32
            B[a, k, np.arange(128)[:, None], np.arange(32)[None, :] * 12 + t[:, None]] = Wa[c, :]
    return B.astype(bf16)


_CACHE = {}
LAST_RESULTS = None
LAST_NC = None
LAST_NCWS = None
LAST_INMAPS = None


def _get_program(ncws, rep=1, ablate=(), ges=None):
    """Build (and cache) the Bass program for given per-adjacency window
    chunk counts. ncws: tuple of 4 tuples of NWIN ints. rep>1 repeats the
    pipeline (device-time measurement via slope); ablate disables stages
    for profiling ("gatheronly", "nogather", "nowphase", "nostbuild").
    ges: gather elem_size in elements (timing experiments; only correct
    when == XW, the default).
    """
    if ges is None:
        ges = XW
    gblk = int(os.environ.get('K_GBLK', 8))
    gbufs = int(os.environ.get('K_GBUFS', 12))
    spkt = os.environ.get('K_SP', '0') == '1'
    key = (ncws, rep, tuple(sorted(ablate)), ges, gblk, gbufs, spkt)
    if key in _CACHE:
        return _CACHE[key]

    import concourse.mybir as mybir
    import concourse.tile as tile
    from concourse import bacc

    DT = (mybir.dt.float16 if os.environ.get("K_DT", "fp16") == "fp16"
          else mybir.dt.bfloat16)
    XDT = mybir.dt.float8e3 if XDT_S == "fp8e3" else DT
    nchs = [int(sum(ncw[0])) for ncw in ncws]
    max_nch = max(int(v) for ncw in ncws for v in ncw[0])

    nc = bacc.Bacc("TRN2", target_bir_lowering=False, debug=False,
                   num_devices=NCORES, num_swdge_queues=4)
    x_d = nc.dram_tensor("xrows", [N, ges], XDT, kind="ExternalInput")
    gi_d = [nc.dram_tensor(f"gi_{a}", [128, nchs[ai] * 8], mybir.dt.int16,
                           kind="ExternalInput") for ai, a in enumerate(ADJS)]
    dew_d = [nc.dram_tensor(f"dew_{a}", [128, 2, nchs[ai]], DT,
                            kind="ExternalInput") for ai, a in enumerate(ADJS)]
    B_d = nc.dram_tensor("B", [4, 3, 128, ES], DT, kind="ExternalInput")
    bias_d = nc.dram_tensor("biasrep", [128, ES], mybir.dt.float32, kind="ExternalInput")
    iotaN_d = nc.dram_tensor("iotaN", [128, 128 * max_nch], DT, kind="ExternalInput")
    out_d = nc.dram_tensor("out", [NDP, ES], DT, kind="ExternalOutput")

    gq = [0]  # gather queue rotation counter
    GBLK = gblk   # chunks per gather block
    GBUFS = gbufs  # msg buffers in flight

    with tile.TileContext(nc) as tc:
        with tc.tile_pool(name="const", bufs=1) as cpool, \
             tc.tile_pool(name="work", bufs=3) as wpool, \
             tc.tile_pool(name="yt", bufs=2) as ytpool, \
             tc.tile_pool(name="outsb", bufs=2) as opool, \
             tc.tile_pool(name="psagg", bufs=4, space="PSUM") as ps_agg, \
             tc.tile_pool(name="pstp", bufs=2, space="PSUM") as ps_tp, \
             tc.tile_pool(name="psout", bufs=2, space="PSUM") as ps_out:

            B_t = cpool.tile([128, 4, 3, ES], DT)
            nc.sync.dma_start(out=B_t[:], in_=B_d.ap().rearrange("a k p e -> p a k e"))
            bias_t = cpool.tile([128, ES], mybir.dt.float32)
            nc.sync.dma_start(out=bias_t[:], in_=bias_d.ap())
            iotaN_t = cpool.tile([128, 128, max_nch], DT)
            nc.sync.dma_start(out=iotaN_t[:], in_=iotaN_d.ap().rearrange(
                "p (j c) -> p j c", c=max_nch))
            from concourse.masks import make_identity
            ident = cpool.tile([128, 128], DT)
            make_identity(nc, ident[:])
            dew_t = []
            gi_t = []
            for ai, a in enumerate(ADJS):
                dt_ = cpool.tile([128, 2, nchs[ai]], DT, name=f"dewt_{a}")
                nc.sync.dma_start(out=dt_[:], in_=dew_d[ai].ap())
                dew_t.append(dt_)
                gt = cpool.tile([128, nchs[ai] * 8], mybir.dt.int16, name=f"git_{a}")
                nc.sync.dma_start(out=gt[:], in_=gi_d[ai].ap())
                gi_t.append(gt)

            # zero all rotating msg buffers once: pad idx slots skip the
            # gather, so those rows must hold finite bytes (fp8 garbage can
            # decode to NaN/Inf and 0*NaN would poison the PSUM accumulate)
            for _ in range(GBUFS):
                m0 = wpool.tile([128, GBLK, ges], XDT, tag="msg", bufs=GBUFS)
                nc.vector.memset(m0[:], 0.0)

            last_msg = [None]
            for _rep in range(rep):
                choffs = [0, 0, 0, 0]
                for w in range(NWIN):
                    yTs = []
                    for ai in range(4):
                        nch = int(ncws[ai][0][w])
                        vw = int(ncws[ai][1][w])
                        choff = choffs[ai]
                        # banded matrix build, transposed layout
                        # stT[e, j, c] = (dlc[e,c] == j) * ew[e,c]
                        stT = wpool.tile([128, 128, max_nch], DT, tag="st", bufs=6)
                        if "nostbuild" in ablate and "gatheronly" not in ablate:
                            nc.vector.tensor_copy(out=stT[:, :, :nch],
                                                  in_=iotaN_t[:, :, :nch])
                        if "nostbuild" not in ablate and "gatheronly" not in ablate:
                            dlc_bT = dew_t[ai][:, 0, choff:choff + nch] \
                                .rearrange("p (o c) -> p o c", o=1) \
                                .to_broadcast([128, 128, nch])
                            ewc_bT = dew_t[ai][:, 1, choff:choff + nch] \
                                .rearrange("p (o c) -> p o c", o=1) \
                                .to_broadcast([128, 128, nch])
                            nc.vector.tensor_tensor(
                                out=stT[:, :, :nch], in0=iotaN_t[:, :, :nch],
                                in1=dlc_bT, op=mybir.AluOpType.is_equal)
                            nc.vector.tensor_tensor(
                                out=stT[:, :, :nch], in0=stT[:, :, :nch],
                                in1=ewc_bT, op=mybir.AluOpType.mult)
                        # gather + aggregation matmuls (y[d, f] in PSUM)
                        psy = ps_agg.tile([128, ES], mybir.dt.float32,
                                          tag="agg")
                        for b0 in range(0, nch, GBLK):
                            bn = min(GBLK, nch - b0)
                            msg = wpool.tile([128, GBLK, ges], XDT,
                                             tag="msg", bufs=GBUFS)
                            if "nogather" not in ablate:
                                nvalid = max(0, min(bn * 128, vw - b0 * 128))
                                nc.gpsimd.dma_gather(
                                    msg[:, :bn, :], x_d.ap(),
                                    gi_t[ai][:, (choff + b0) * 8:(choff + b0 + bn) * 8],
                                    bn * 128, nvalid, ges, elem_step=ges,
                                    single_packet=spkt, queue_num=gq[0] % 4)
                                gq[0] += 1
                                last_msg[0] = msg
                            else:
                                nc.vector.memset(msg[:, :bn, 0:2], 0.0)
                            if "gatheronly" in ablate:
                                continue
                            for jj in range(bn):
                                ch = b0 + jj
                                nc.tensor.matmul(
                                    psy[:, :],
                                    lhsT=stT[:, :, ch],
                                    rhs=msg[:, jj, :ES],
                                    start=(ch == 0), stop=(ch == nch - 1))
                        choffs[ai] = choff + nch
                        if "gatheronly" in ablate:
                            continue
                        ysb = ytpool.tile([128, ES], DT, tag=f"ysb{ai}")
                        nc.scalar.copy(out=ysb[:], in_=psy[:])
                        yTs.append(ysb)
                    # ---- W transform for this window ----
                    if "nowphase" in ablate or "gatheronly" in ablate:
                        continue
                    yTt = []
                    for i in range(12):
                        ai, k = divmod(i, 3)
                        pst = ps_tp.tile([128, 128], DT, tag="tp")
                        nc.tensor.transpose(
                            pst[:], yTs[ai][:, 128 * k:128 * (k + 1)], ident[:])
                        yT = ytpool.tile([128, 128], DT, tag="yT", bufs=13)
                        if i % 2 == 0:
                            nc.vector.tensor_copy(out=yT[:], in_=pst[:])
                        else:
                            nc.scalar.copy(out=yT[:], in_=pst[:])
                        yTt.append(yT)
                    pso = ps_out.tile([128, ES], mybir.dt.float32, tag="wout")
                    for i in range(12):
                        ai, k = divmod(i, 3)
                        nc.tensor.matmul(pso[:], lhsT=yTt[i][:],
                                         rhs=B_t[:, ai, k, :],
                                         start=(i == 0), stop=(i == 11))
                    outsb = opool.tile([128, ES], DT, tag="out")
                    nc.vector.tensor_tensor(out=outsb[:], in0=pso[:], in1=bias_t[:],
                                            op=mybir.AluOpType.add)
                    nc.sync.dma_start(out=out_d.ap()[128 * w:128 * (w + 1), :],
                                      in_=outsb[:])
                if "gatheronly" in ablate:
                    # chain program completion to the last gather
                    outsb = opool.tile([128, ES], DT, tag="out")
                    nc.vector.tensor_copy(out=outsb[:, 0:GBLK * ES // 128],
                                          in_=last_msg[0][:, 0, 0:GBLK * ES // 128])
                    nc.sync.dma_start(out=out_d.ap()[0:128, :], in_=outsb[:])

    nc.compile()
    _CACHE[key] = nc
    return nc


def _default_mapping():
    """Original contiguous dst sharding: core n//ND, window (n%ND)//WIN."""
    nid = np.arange(N)
    core = nid // ND
    r = nid % ND
    return (core * NWIN + r // WIN).astype(np.int32), (r % WIN).astype(np.int32)


def _host_prep(x, Ws, bias, eis, ews):
    xr = np.asarray(x).astype(np.float32).transpose(0, 2, 1).reshape(N, ES)
    if XDT_S == "fp8e3":
        x_rows = np.zeros((N, XW), x8t)
        x_rows[:, :ES] = xr.astype(x8t)
    else:
        x_rows = np.ascontiguousarray(xr).astype(bf16)
    if os.environ.get("K_BAL", "1") == "1":
        binof, colof = _dst_mapping(eis)
    else:
        binof, colof = _default_mapping()
    ncws, gis, dlcs = [], [], []
    for a in ADJS:
        pc = _prep_edges(np.asarray(eis[a]), np.asarray(ews[a]), binof, colof)
        nv, gi, dew = _build_adj_inputs(pc)
        ncws.append((tuple(int(v) for v in nv[0]), tuple(int(v) for v in nv[1])))
        gis.append(gi)
        dlcs.append(dew)
    B = _build_B(Ws)
    bias_rep = np.ascontiguousarray(
        np.tile(np.repeat(np.asarray(bias).astype(np.float32), T)[None, :], (128, 1)))
    max_nch = max(int(v) for ncw in ncws for v in ncw[0])
    iotaN = np.ascontiguousarray(np.broadcast_to(
        np.repeat(np.arange(128, dtype=np.float32), max_nch)[None, :],
        (128, 128 * max_nch))).astype(bf16)
    in_maps = []
    for k in range(NCORES):
        m = {"xrows": x_rows, "B": B, "biasrep": bias_rep, "iotaN": iotaN}
        for ai, a in enumerate(ADJS):
            m[f"gi_{a}"] = gis[ai][k]
            m[f"dew_{a}"] = dlcs[ai][k]
        in_maps.append(m)
    return tuple(ncws), in_maps, (binof, colof)


def kernel(x, W_fwd1, W_fwd2, W_bck1, W_bck2, bias,
           ew_fwd1, ew_fwd2, ew_bck1, ew_bck2,
           ei_fwd1, ei_fwd2, ei_bck1, ei_bck2):
    from concourse.bass_utils import run_bass_kernel_spmd

    x = np.asarray(x)
    eis = dict(fwd1=ei_fwd1, fwd2=ei_fwd2, bck1=ei_bck1, bck2=ei_bck2)
    ews = dict(fwd1=ew_fwd1, fwd2=ew_fwd2, bck1=ew_bck1, bck2=ew_bck2)
    Ws = [W_fwd1, W_fwd2, W_bck1, W_bck2]

    ncws, in_maps, (binof, colof) = _host_prep(x, Ws, bias, eis, ews)
    nc = _get_program(ncws)

    res = run_bass_kernel_spmd(nc, in_maps, core_ids=list(range(NCORES)))
    global LAST_RESULTS, LAST_NC, LAST_INMAPS, LAST_NCWS
    LAST_RESULTS = res
    LAST_NC = nc
    LAST_INMAPS = in_maps
    LAST_NCWS = ncws

    # un-permute: node n lives at row (binof%NWIN)*WIN + colof of core binof//NWIN
    stacked = np.stack([np.asarray(res.results[k]["out"][:NDP])
                        for k in range(NCORES)], 0)       # [8, NDP, 384]
    rows = (binof % NWIN) * WIN + colof
    out = stacked[binof // NWIN, rows].astype(np.float32).reshape(N, C, T)
    return out

